# revision 3
# baseline (speedup 1.0000x reference)
"""Trainium2 Bass kernel: scatter-add of table rows into a voxel grid.

Arch-Y: identity-matmul accumulate, round-major slot streams.
  - Host: sort events by cell; snake-deal cells to 8 cores by count;
    per core sort cells by count desc, tile 128 consecutive cells.
    Tile t needs K_t = max event-count in tile rounds; SPMD schedule
    S[t] = max over cores. Slot stream is round-major: for each tile,
    round k holds cell j's k-th event at chunk position j (zero-row
    pad if the cell has < k events).
  - Device: dma_gather 1024 rows/call (bf16, 4 SWDGE queues); each
    128-slot chunk is accumulated into the tile's PSUM tile by a
    matmul with a constant fp8 identity (no per-chunk weight loads,
    no one-hot builds); PSUM -> bf16 stage -> DMA out.
  - Host: reassemble [B,W,H,L,D] fp32 from per-core bf16 outputs.
"""

import numpy as np
import ml_dtypes

B, W, H, L, D = 4, 32, 32, 32, 256
NCELLS = B * W * H * L          # 131072
TROWS = 4096
TROWS_PAD = TROWS + 128         # 128 zero rows for pad slots
NCORES = 8
CPT = 128                       # cells per tile
TPC = NCELLS // CPT // NCORES   # tiles per core: 128
GIDX = 1024                     # idxs per dma_gather call (HW ring limit)
GCH = GIDX // 128               # chunks per gather call: 8

_compiled = {}


def _build(S):
    import concourse.tile as tile
    from concourse import bacc, mybir

    f32, bf16, i16 = mybir.dt.float32, mybir.dt.bfloat16, mybir.dt.int16
    f8 = mybir.dt.float8e4
    nch = int(sum(S))
    assert nch % GCH == 0
    ncalls = nch // GCH

    nc = bacc.Bacc("TRN2", target_bir_lowering=False, debug=False,
                   num_devices=NCORES, num_swdge_queues=4)
    tabbf = nc.dram_tensor("tabbf", [TROWS_PAD, D], bf16, kind="ExternalInput")
    rows_w = nc.dram_tensor("rows_w", [128, ncalls * (GIDX // 16)], i16,
                            kind="ExternalInput")
    ident = nc.dram_tensor("ident", [128, 128], f8, kind="ExternalInput")
    out = nc.dram_tensor("out", [TPC * 128, D], bf16, kind="ExternalOutput")

    with tile.TileContext(nc) as tc:
        with tc.tile_pool(name="const", bufs=1) as constp, \
             tc.tile_pool(name="gbuf", bufs=18) as gpool, \
             tc.tile_pool(name="psum", bufs=8, space="PSUM") as pspool, \
             tc.tile_pool(name="stage", bufs=4) as stpool:
            id_sb = constp.tile([128, 128], f8)
            nc.sync.dma_start(id_sb[:], ident[:])
            # idx stream: load in 8-call segments so call 0 starts early
            rows_sb = constp.tile([128, ncalls * (GIDX // 16)], i16)
            SEG = 8 * (GIDX // 16)
            nseg = -(-(ncalls * (GIDX // 16)) // SEG)
            for si in range(nseg):
                lo, hi = si * SEG, min((si + 1) * SEG, ncalls * (GIDX // 16))
                nc.sync.dma_start(rows_sb[:, lo:hi], rows_w[:, lo:hi])

            OB = 4
            gt = None
            st = None
            c = 0
            for t in range(TPC):
                ps = pspool.tile([128, D], f32, space="PSUM")
                K = int(S[t])
                for j in range(K):
                    if c % GCH == 0:
                        ci = c // GCH
                        gt = gpool.tile([128, GCH, D], bf16)
                        nc.gpsimd.dma_gather(
                            gt[:], tabbf[:],
                            rows_sb[:, ci * (GIDX // 16):(ci + 1) * (GIDX // 16)],
                            GIDX, GIDX, D, queue_num=ci % 4,
                            single_packet=False)
                    nc.tensor.matmul(out=ps[:], lhsT=id_sb[:],
                                     rhs=gt[:, c % GCH, :],
                                     start=(j == 0), stop=(j == K - 1))
                    c += 1
                if t % OB == 0:
                    st = stpool.tile([128, OB, D], bf16)
                nc.scalar.copy(st[:, t % OB, :], ps[:])
                if t % OB == OB - 1:
                    t0 = t - (OB - 1)
                    nc.sync.dma_start(
                        out[t0 * 128:(t0 + OB) * 128, :].rearrange(
                            "(q p) d -> p q d", p=128),
                        st[:])
            assert c == nch
    nc.compile()
    return nc


def _marshal(event_cell, event_row):
    """Sort events by cell; deal cells to cores; count-sorted tiles;
    round-major slot streams with zero-row pads."""
    ecell = np.asarray(event_cell).astype(np.int64)
    erow = np.asarray(event_row).astype(np.int64)
    order = np.argsort(ecell, kind="stable")
    scell = ecell[order]
    srow = erow[order].astype(np.int32)
    bounds = np.searchsorted(scell, np.arange(NCELLS + 1))
    counts = np.diff(bounds).astype(np.int64)

    # snake-deal cells (sorted by count desc) to cores
    deal = np.argsort(-counts, kind="stable")
    r = np.arange(NCELLS) % (2 * NCORES)
    core_of_rank = np.where(r < NCORES, r, 2 * NCORES - 1 - r)
    # per core: list of cells sorted by count desc (deal order is desc)
    core_cells = [deal[core_of_rank == cidx] for cidx in range(NCORES)]
    for cc in core_cells:
        assert len(cc) == TPC * CPT

    # per-tile K per core; SPMD schedule S = positionwise max
    Kmat = np.zeros((NCORES, TPC), np.int64)
    for cidx in range(NCORES):
        cc = core_cells[cidx]
        Kmat[cidx] = counts[cc].reshape(TPC, CPT).max(axis=1)
    S = Kmat.max(axis=0)
    S = np.maximum(S, 1)
    # pad total chunks to a multiple of GCH
    S[-1] += (-int(S.sum())) % GCH
    nch = int(S.sum())

    in_maps = []
    for cidx in range(NCORES):
        cc = core_cells[cidx]
        cnt = counts[cc].reshape(TPC, CPT)
        st = bounds[cc].reshape(TPC, CPT)
        stream = np.empty(nch * 128, np.int32)
        pos = 0
        for t in range(TPC):
            K = int(S[t])
            # rounds x cells: srow[st + k] where k < cnt else pad
            kk = np.arange(K)[:, None]                    # [K, 1]
            idx = st[t][None, :] + kk                     # [K, CPT]
            valid = kk < cnt[t][None, :]
            blk = np.where(valid, srow[np.minimum(idx, len(srow) - 1)], 0)
            padpos = TROWS + (np.arange(K * CPT).reshape(K, CPT) + pos) % 128
            blk = np.where(valid, blk, padpos)
            stream[pos:pos + K * CPT] = blk.reshape(-1)
            pos += K * CPT
        assert pos == nch * 128
        # wrap by 16, replicate to 128 partitions
        wr = stream.reshape(-1, GIDX).reshape(-1, GIDX // 16, 16)
        wr = wr.transpose(0, 2, 1).reshape(-1, 16, GIDX // 16)
        wr = np.concatenate(list(wr), axis=1)
        wr = np.tile(wr, (8, 1)).astype(np.int16)
        in_maps.append({"rows_w": np.ascontiguousarray(wr)})
    return in_maps, tuple(int(x) for x in S), core_cells


def kernel(table, event_cell, event_row, _want_trace=False):
    from concourse.bass_utils import run_bass_kernel_spmd

    tabbf = np.zeros((TROWS_PAD, D), dtype=ml_dtypes.bfloat16)
    tabbf[:TROWS] = np.asarray(table, dtype=np.float32).astype(
        ml_dtypes.bfloat16)
    identv = np.eye(128, dtype=ml_dtypes.float8_e4m3)
    in_maps, S, core_cells = _marshal(event_cell, event_row)
    for m in in_maps:
        m["tabbf"] = tabbf
        m["ident"] = identv

    if S not in _compiled:
        _compiled[S] = _build(S)
    nc = _compiled[S]

    kw = {"trace": True} if _want_trace else {}
    res = run_bass_kernel_spmd(nc, in_maps, core_ids=list(range(NCORES)), **kw)
    full = np.empty((NCELLS, D), np.float32)
    for cidx in range(NCORES):
        co = np.asarray(res.results[cidx]["out"]).astype(np.float32)
        full[core_cells[cidx]] = co
    out = full.reshape(B, W, H, L, D)
    if _want_trace:
        return out, res
    return out
